# revision 2
# baseline (speedup 1.0000x reference)
import math
import os
import sys

import numpy as np

# nn_AxialAttentionD: B,C,D,H,W = 1,64,48,64,128; 4 heads, head_dim 16.
# Attention over D independently per (head, h, w). Sharded over H across
# 8 NeuronCores (8 H-rows per core). Bass/Tile kernel; per-core pipeline:
#   chunk = 64 spatial positions (half an H-row), cols = (d, w) d-major
#   1. DMA x chunk [64, 3072] fp32
#   2. QKV GEMMs (f32r weights/rhs) -> padded q/k (4h x 32 rows: 16 data
#      + 16 zeros) in bf16, dense v bf16
#   3. v relayout SBUF->SBUF DMA into [48(d), (4h x 32)(c|ones) * w]
#   4. per position: 4 row-tiled QK matmuls (K=32 blocks) -> T^T psum
#   5. batched exp (ACT) with 1/4 scale -> bf16
#   6. per position: 4 col-tiled AV matmuls with ones-cols -> O + Z rows
#   7. batched divide (normalize) -> padded O bf16
#   8. proj GEMM (zero-padded weights kill Z/junk rows) -> fp32 out
#   9. DMA out
# The harness imports kernel() below; heavy deps load lazily so the
# module imports even off-device.

sys.path.insert(0, "/opt/trn_rl_repo")

NUM_HEADS = 4
C = 64
D = 48
H = 64
W = 128
DIM = 16
N_CORES = 8
HSH = H // N_CORES          # H rows per core shard
W_C = 64                    # positions per chunk (half an H row)
N_C = D * W_C               # cols per chunk (3072)
POS_GROUP = 8               # positions per AV psum group
EXP_GROUP = 8               # positions per exp batch


def _sinusoidal_pe(dim: int, depth: int) -> np.ndarray:
    half = (dim + 1) // 2
    inv_freq = np.exp(
        np.arange(half, dtype=np.float32) * (-math.log(10000.0) / max(1, half - 1))
    )
    pos = np.arange(depth, dtype=np.float32)
    angles = pos[:, None] * inv_freq[None, :]
    sin = np.sin(angles).T.astype(np.float32)
    cos = np.cos(angles).T.astype(np.float32)
    pe = np.zeros((dim, depth), dtype=np.float32)
    even = dim // 2
    if even > 0:
        pe[0 : 2 * even : 2, :] = sin[:even]
        pe[1 : 2 * even : 2, :] = cos[:even]
    if dim % 2 == 1:
        pe[-1, :] = sin[-1]
    return pe


def _prep_weights(qkv_w: np.ndarray, proj_w: np.ndarray):
    """Host-side: pad + transpose weights for the device layouts."""
    wq = qkv_w[0:C, :]            # [64, 64] rows (h,cc)
    wk = qkv_w[C : 2 * C, :]
    wv = qkv_w[2 * C : 3 * C, :]
    # lhsT for q/k passes: [64 (x-ch), 128 (pad rows 32h+cc)]
    wqT_pad = np.zeros((C, 128), dtype=np.float32)
    wkT_pad = np.zeros((C, 128), dtype=np.float32)
    for h in range(NUM_HEADS):
        wqT_pad[:, 32 * h : 32 * h + 16] = wq[h * 16 : h * 16 + 16, :].T
        wkT_pad[:, 32 * h : 32 * h + 16] = wk[h * 16 : h * 16 + 16, :].T
    wvT = np.ascontiguousarray(wv.T)  # [64, 64]
    # proj lhsT: [128 (pad rows 32h+cc), 64 (c_out)], zeros at 32h+16+*
    wpT_pad = np.zeros((128, C), dtype=np.float32)
    for h in range(NUM_HEADS):
        wpT_pad[32 * h : 32 * h + 16, :] = proj_w[:, h * 16 : h * 16 + 16].T
    # pe tile materialized in chunk layout [128 pad rows, N_C]
    pe = _sinusoidal_pe(DIM, D)  # [16, 48]
    pe_t = np.zeros((128, N_C), dtype=np.float32)
    for h in range(NUM_HEADS):
        for cc in range(DIM):
            pe_t[32 * h + cc, :] = np.repeat(pe[cc, :], W_C)
    return wqT_pad, wkT_pad, wvT, wpT_pad, pe_t


def build_bass(hsh: int = HSH):
    import concourse.bass as bass
    import concourse.mybir as mybir
    from concourse import tile

    f32 = mybir.dt.float32
    f32r = mybir.dt.float32r
    bf16 = mybir.dt.bfloat16

    nc = bass.Bass("TRN2", target_bir_lowering=False, debug=False)

    x_in = nc.dram_tensor("x", [C, D, hsh, W], f32, kind="ExternalInput")
    wq_d = nc.dram_tensor("wqT_pad", [C, 128], f32, kind="ExternalInput")
    wk_d = nc.dram_tensor("wkT_pad", [C, 128], f32, kind="ExternalInput")
    wv_d = nc.dram_tensor("wvT", [C, C], f32, kind="ExternalInput")
    wp_d = nc.dram_tensor("wpT_pad", [128, C], f32, kind="ExternalInput")
    pe_d = nc.dram_tensor("pe_t", [128, N_C], f32, kind="ExternalInput")
    y_out = nc.dram_tensor("y", [C, D, hsh, W], f32, kind="ExternalOutput")

    n_chunks = hsh * (W // W_C)
    n_slices = N_C // 512  # 6 GEMM N-slices per chunk

    with tile.TileContext(nc) as tc:
        with (
            tc.tile_pool(name="const", bufs=1) as constp,
            tc.tile_pool(name="xin", bufs=2) as xp,
            tc.tile_pool(name="qk", bufs=2) as qkp,
            tc.tile_pool(name="vd", bufs=2) as vp,
            tc.tile_pool(name="vv", bufs=2) as vvp,
            tc.tile_pool(name="texp", bufs=3) as texpp,
            tc.tile_pool(name="opad", bufs=2) as opadp,
            tc.tile_pool(name="yo", bufs=2) as yop,
            tc.tile_pool(name="gemm_ps", bufs=2, space="PSUM") as gpsp,
            tc.tile_pool(name="t_ps", bufs=2, space="PSUM") as tpsp,
            tc.tile_pool(name="o_ps", bufs=2, space="PSUM") as opsp,
        ):
            # ---- constants: load + cast weights to bf16 ----
            wq_f = constp.tile([C, 128], f32, tag="wq_f")
            wk_f = constp.tile([C, 128], f32, tag="wk_f")
            wv_f = constp.tile([C, C], f32, tag="wv_f")
            wp_f = constp.tile([128, C], f32, tag="wp_f")
            pe_sb = constp.tile([128, N_C], f32, tag="pe")
            nc.sync.dma_start(wq_f[:], wq_d.ap())
            nc.sync.dma_start(wk_f[:], wk_d.ap())
            nc.sync.dma_start(wv_f[:], wv_d.ap())
            nc.sync.dma_start(wp_f[:], wp_d.ap())
            nc.sync.dma_start(pe_sb[:], pe_d.ap())
            wp_b = constp.tile([128, C], bf16, tag="wp_b")
            nc.vector.tensor_copy(wp_b[:], wp_f[:])

            # persistent double-buffered tiles that carry constant regions
            vv_tiles = []
            opad_tiles = []
            for i in range(2):
                vv = vvp.tile([D, 128 * W_C], bf16, tag=f"vv{i}")
                # ones in cols (32h+16+k)*W_C .. for the Z rows
                vvr = vv[:].rearrange("p (c w) -> p c w", w=W_C)
                for h in range(NUM_HEADS):
                    nc.vector.memset(vvr[:, 32 * h + 16 : 32 * h + 32, :], 1.0)
                vv_tiles.append(vv)
                op = opadp.tile([128, N_C], bf16, tag=f"opad{i}")
                nc.vector.memset(op[112:128, :], 0.0)
                opad_tiles.append(op)

            for ci in range(n_chunks):
                hs = ci // (W // W_C)
                wb = ci % (W // W_C)
                vv = vv_tiles[ci % 2]
                opad = opad_tiles[ci % 2]

                # ---- 1. DMA x chunk ----
                x_t = xp.tile([C, D, W_C], f32, tag="x")
                nc.sync.dma_start(
                    x_t[:], x_in.ap()[:, :, hs, wb * W_C : (wb + 1) * W_C]
                )
                x_flat = x_t[:].rearrange("p d w -> p (d w)")

                # ---- 2. QKV GEMMs ----
                q_t = qkp.tile([128, N_C], bf16, tag="q")
                k_t = qkp.tile([128, N_C], bf16, tag="k")
                v_t = vp.tile([C, N_C], bf16, tag="v")
                for si in range(n_slices):
                    sl = slice(si * 512, (si + 1) * 512)
                    rhs = x_flat[:, sl].bitcast(f32r)
                    ps_q = gpsp.tile([128, 512], f32, tag="gemm")
                    nc.tensor.matmul(
                        ps_q[:], wq_f[:].bitcast(f32r), rhs, start=True, stop=True
                    )
                    nc.vector.tensor_tensor(
                        q_t[:, sl], ps_q[:], pe_sb[:, sl], mybir.AluOpType.add
                    )
                    ps_k = gpsp.tile([128, 512], f32, tag="gemm")
                    nc.tensor.matmul(
                        ps_k[:], wk_f[:].bitcast(f32r), rhs, start=True, stop=True
                    )
                    nc.vector.tensor_tensor(
                        k_t[:, sl], ps_k[:], pe_sb[:, sl], mybir.AluOpType.add
                    )
                    ps_v = gpsp.tile([128, 512], f32, tag="gemm")
                    nc.tensor.matmul(
                        ps_v[0:C, :], wv_f[:].bitcast(f32r), rhs, start=True, stop=True
                    )
                    nc.vector.tensor_copy(v_t[:, sl], ps_v[0:C, :])

                # ---- 3. v relayout: [64,(d,w)] -> [48(d), (4h x 32)(c)*w] ----
                vvr = vv[:].rearrange("p (c w) -> p c w", w=W_C)
                for h in range(NUM_HEADS):
                    src = v_t[16 * h : 16 * h + 16, :].rearrange(
                        "c (d w) -> d c w", w=W_C
                    )
                    nc.sync.dma_start(vvr[:, 32 * h : 32 * h + 16, :], src)

                qr = q_t[:].rearrange("p (d w) -> p d w", w=W_C)
                kr = k_t[:].rearrange("p (d w) -> p d w", w=W_C)
                opr = opad[:].rearrange("p (d w) -> p d w", w=W_C)

                # ---- 4-7. attention, EXP_GROUP positions at a time ----
                for g in range(W_C // EXP_GROUP):
                    t_ps = tpsp.tile([112, 4 * D * EXP_GROUP // 2], f32, tag="t")
                    # QK: per position 4 row-tiled matmuls K=32
                    for pi in range(EXP_GROUP):
                        p = g * EXP_GROUP + pi
                        rbase = 64 * (pi % 2)
                        cbase = (pi // 2) * 192
                        for h in range(NUM_HEADS):
                            nc.tensor.matmul(
                                t_ps[rbase : rbase + D, cbase + 48 * h : cbase + 48 * h + 48],
                                kr[32 * h : 32 * h + 32, :, p],
                                qr[32 * h : 32 * h + 32, :, p],
                                start=True,
                                stop=True,
                                tile_position=(32 * h, rbase),
                            )
                    # 5. exp (scale = 1/sqrt(dim) = 0.25)
                    te = texpp.tile([112, 4 * D * EXP_GROUP // 2], bf16, tag="te")
                    nc.scalar.activation(
                        te[:], t_ps[:], mybir.ActivationFunctionType.Exp, scale=0.25
                    )
                    # 6. AV: per position 4 col-tiled matmuls + ones rows
                    o_ps = opsp.tile([128, D * EXP_GROUP], f32, tag="o")
                    for pi in range(EXP_GROUP):
                        p = g * EXP_GROUP + pi
                        rbase = 64 * (pi % 2)
                        cbase = (pi // 2) * 192
                        for h in range(NUM_HEADS):
                            nc.tensor.matmul(
                                o_ps[32 * h : 32 * h + 32, 48 * pi : 48 * pi + 48],
                                vvr[:, 32 * h : 32 * h + 32, p],
                                te[rbase : rbase + D, cbase + 48 * h : cbase + 48 * h + 48],
                                start=True,
                                stop=True,
                                tile_position=(0, 32 * h),
                            )
                    # 7. normalize: O / Z via 16-shifted partition AP
                    o_iw = o_ps[:].rearrange("p (g i) -> p i g", i=D)
                    nc.vector.tensor_tensor(
                        opr[0:112, :, g * EXP_GROUP : (g + 1) * EXP_GROUP],
                        o_iw[0:112],
                        o_iw[16:128],
                        mybir.AluOpType.divide,
                    )

                # ---- 8. proj + out ----
                y_sb = yop.tile([C, N_C], f32, tag="y")
                for si in range(n_slices):
                    sl = slice(si * 512, (si + 1) * 512)
                    ps_y = gpsp.tile([128, 512], f32, tag="gemm")
                    nc.tensor.matmul(
                        ps_y[0:C, :], wp_b[:], opad[:, sl], start=True, stop=True
                    )
                    nc.vector.tensor_copy(y_sb[:, sl], ps_y[0:C, :])
                nc.sync.dma_start(
                    y_out.ap()[:, :, hs, wb * W_C : (wb + 1) * W_C],
                    y_sb[:].rearrange("p (d w) -> p d w", w=W_C),
                )

    return nc


def _run_hw(x: np.ndarray, qkv_w: np.ndarray, proj_w: np.ndarray) -> np.ndarray:
    from concourse.bass_utils import run_bass_kernel_spmd

    B = x.shape[0]
    assert x.shape == (B, C, D, H, W)
    wqT_pad, wkT_pad, wvT, wpT_pad, pe_t = _prep_weights(qkv_w, proj_w)
    nc = build_bass(HSH)

    out = np.empty_like(x)
    for b in range(B):
        in_maps = []
        for core in range(N_CORES):
            xs = np.ascontiguousarray(
                x[b, :, :, core * HSH : (core + 1) * HSH, :]
            )
            in_maps.append(
                {
                    "x": xs,
                    "wqT_pad": wqT_pad,
                    "wkT_pad": wkT_pad,
                    "wvT": wvT,
                    "wpT_pad": wpT_pad,
                    "pe_t": pe_t,
                }
            )
        res = run_bass_kernel_spmd(nc, in_maps, list(range(N_CORES)))
        for core in range(N_CORES):
            out[b, :, :, core * HSH : (core + 1) * HSH, :] = res.results[core]["y"]
    return out


def kernel(x: np.ndarray, qkv_w: np.ndarray, proj_w: np.ndarray) -> np.ndarray:
    x = np.asarray(x, dtype=np.float32)
    qkv_w = np.asarray(qkv_w, dtype=np.float32)
    proj_w = np.asarray(proj_w, dtype=np.float32)
    return _run_hw(x, qkv_w, proj_w)
